# revision 14
# baseline (speedup 1.0000x reference)
"""Trainium2 Bass kernel for the quantized ResNet Bottleneck block.

Strategy
--------
Data parallel over batch: 64 images -> 8 cores x 8 images. Each core runs an
identical Bass program; weights are replicated.

All convs are executed as integer-valued bf16 matmuls accumulated in fp32
PSUM (exact: quantized codes are integers in [-127,127]; we add +128 so
activations live in [1,255], still exact bf16; the +128 offset passes through
each conv as a per-output-channel constant which is folded into the next
bias on the host).

Quantization rounding is done with two exact fp32 tricks:
  * x-quant: magic-constant add (C = 1.5*2^23) -> RNE round-to-integer.
  * conv epilogues: ReLU clamp on ACT, then (min 127) + 128 and convert
    fp32->bf16; in the [128,256) binade bf16 RNE == round-half-even.

conv3 (1x1) and the stride-2 shortcut conv accumulate into the same PSUM
tile; shortcut weights are pre-scaled by (css/c3s) on the host so both
contributions share one output scale.
"""

import sys
from contextlib import ExitStack

import numpy as np

sys.path.insert(0, "/opt/trn_rl_repo")

import ml_dtypes  # noqa: E402

import concourse.bacc as bacc  # noqa: E402
import concourse.bass as bass  # noqa: E402
import concourse.tile as tile  # noqa: E402
from concourse import mybir  # noqa: E402
from concourse.bass_utils import run_bass_kernel_spmd  # noqa: E402

F32 = mybir.dt.float32
BF16 = mybir.dt.bfloat16
ALU = mybir.AluOpType
AFT = mybir.ActivationFunctionType
BF16NP = ml_dtypes.bfloat16

C_MAGIC = float(np.float32(12582912.0))  # 1.5 * 2**23

N_CORES = 8
B_LOC = 8  # images per core
PAIRS = B_LOC // 2

# Use one matmul per image pair (4D rhs access pattern) where possible.
PAIR_MM = True


def _build_nc(pairs=PAIRS, debug_taps=False):
    nc = bacc.Bacc("TRN2", target_bir_lowering=False, debug=False)
    b_loc = 2 * pairs

    x_d = nc.dram_tensor("x", [b_loc, 4, 128, 784], F32, kind="ExternalInput")
    w1_d = nc.dram_tensor("w1l", [4, 128, 256], BF16, kind="ExternalInput")
    w2_d = nc.dram_tensor("w2l", [9, 2, 128, 256], BF16, kind="ExternalInput")
    w3_d = nc.dram_tensor("w3l", [2, 128, 1024], BF16, kind="ExternalInput")
    ws_d = nc.dram_tensor("wsl", [4, 128, 1024], BF16, kind="ExternalInput")
    b1_d = nc.dram_tensor("beta1", [128, 2], F32, kind="ExternalInput")
    b2_d = nc.dram_tensor("beta2", [128, 2], F32, kind="ExternalInput")
    dl_d = nc.dram_tensor("delta", [128, 8], F32, kind="ExternalInput")
    y_d = nc.dram_tensor("y", [b_loc, 8, 128, 196], F32, kind="ExternalOutput")
    if debug_taps:
        xq_d = nc.dram_tensor("dbg_xq", [4, 128, 1568], BF16, kind="ExternalOutput")
        p2_d = nc.dram_tensor("dbg_p2", [2, 128, 1856], BF16, kind="ExternalOutput")
        t3_d = nc.dram_tensor("dbg_t3", [2, 128, 392], BF16, kind="ExternalOutput")

    # scales (filled per-call via globals set by kernel(); see _SCALES)
    a1, a2, g3 = _SCALES

    with tile.TileContext(nc) as tc, ExitStack() as ctx:
        wp = ctx.enter_context(tc.tile_pool(name="w", bufs=1))
        xinp = ctx.enter_context(tc.tile_pool(name="xin", bufs=3))
        yap = ctx.enter_context(tc.tile_pool(name="ya", bufs=3))
        tbp = ctx.enter_context(tc.tile_pool(name="tb", bufs=3))
        xqp = ctx.enter_context(tc.tile_pool(name="xq", bufs=2))
        p2p = ctx.enter_context(tc.tile_pool(name="p2", bufs=2))
        t3p = ctx.enter_context(tc.tile_pool(name="t3", bufs=2))
        rp = ctx.enter_context(tc.tile_pool(name="r", bufs=4))
        yop = ctx.enter_context(tc.tile_pool(name="yo", bufs=4))
        pc1 = ctx.enter_context(tc.tile_pool(name="pc1", bufs=4, space="PSUM"))
        pc2 = ctx.enter_context(tc.tile_pool(name="pc2", bufs=2, space="PSUM"))
        pc3 = ctx.enter_context(tc.tile_pool(name="pc3", bufs=2, space="PSUM"))

        # ---- weights + biases (loaded once) ----
        w1 = []
        for k in range(4):
            t = wp.tile([128, 256], BF16, tag=f"w1_{k}")
            nc.sync.dma_start(t[:], w1_d[k])
            w1.append(t)
        w2 = []
        for tp in range(9):
            row = []
            for k in range(2):
                t = wp.tile([128, 256], BF16, tag=f"w2_{tp}_{k}")
                nc.sync.dma_start(t[:], w2_d[tp, k])
                row.append(t)
            w2.append(row)
        w3 = []
        for k in range(2):
            t = wp.tile([128, 1024], BF16, tag=f"w3_{k}")
            nc.sync.dma_start(t[:], w3_d[k])
            w3.append(t)
        ws = []
        for k in range(4):
            t = wp.tile([128, 1024], BF16, tag=f"ws_{k}")
            nc.sync.dma_start(t[:], ws_d[k])
            ws.append(t)
        beta1 = wp.tile([128, 2], F32, tag="beta1")
        nc.sync.dma_start(beta1[:], b1_d[:])
        beta2 = wp.tile([128, 2], F32, tag="beta2")
        nc.sync.dma_start(beta2[:], b2_d[:])
        delta = wp.tile([128, 8], F32, tag="delta")
        nc.sync.dma_start(delta[:], dl_d[:])

        for p in range(pairs):
            # ---- load x pair + quantize: X' = round(clip(2x,-1,1)*127)+128 ----
            xq = []
            for k in range(4):
                xin = xinp.tile([128, 1568], F32)
                for i in (0, 1):
                    nc.sync.dma_start(
                        xin[:, i * 784 : (i + 1) * 784], x_d[2 * p + i, k]
                    )
                ya = yap.tile([128, 1568], F32)
                nc.gpsimd.tensor_scalar(
                    ya[:], xin[:], 0.5, 254.0, op0=ALU.min, op1=ALU.mult
                )
                tb = tbp.tile([128, 1568], F32)
                nc.vector.tensor_scalar(
                    tb[:], ya[:], C_MAGIC, C_MAGIC - 127.0, op0=ALU.add, op1=ALU.max
                )
                xqk = xqp.tile([128, 1568], BF16, tag=f"xq{k}")
                nc.vector.tensor_scalar(
                    xqk[:], tb[:], C_MAGIC - 128.0, None, op0=ALU.subtract
                )
                xq.append(xqk)
                if debug_taps and p == 0:
                    nc.sync.dma_start(xq_d[k], xqk[:])

            # ---- conv1 (1x1, 512->256) + epilogue into padded tile ----
            p2 = []
            for m in range(2):
                # pad value 128 == quantized zero in the +128-shifted domain
                t = p2p.tile([128, 1856], BF16, tag=f"p2_{m}")
                nc.gpsimd.memset(t[:], 128.0)
                p2.append(t)
            for m in range(2):
                pv = p2[m].rearrange("q (i r c) -> q i r c", i=2, r=29, c=32)
                for i in (0, 1):
                    for hf in (0, 1):
                        ps = pc1.tile([128, 392], F32)
                        off = i * 784 + hf * 392
                        for k in range(4):
                            nc.tensor.matmul(
                                ps[:],
                                w1[k][:, m * 128 : (m + 1) * 128],
                                xq[k][:, off : off + 392],
                                start=(k == 0),
                                stop=(k == 3),
                            )
                        r = rp.tile([128, 392], F32, tag="r1")
                        nc.scalar.activation(
                            r[:], ps[:], AFT.Relu, bias=beta1[:, m : m + 1], scale=a1
                        )
                        nc.vector.tensor_scalar(
                            pv[:, i, 1 + 14 * hf : 15 + 14 * hf, 2:30],
                            r[:].rearrange("q (a b) -> q a b", a=14),
                            127.0,
                            128.0,
                            op0=ALU.min,
                            op1=ALU.add,
                        )

            # ---- conv2 (3x3 s2, 256->256) + epilogue ----
            t3 = []
            for m in range(2):
                ps2 = pc2.tile([128, 392], F32)
                first = True
                for k in range(2):
                    pv = p2[k].rearrange("q (i r c) -> q i r c", i=2, r=29, c=32)
                    for tp in range(9):
                        ky, kx = divmod(tp, 3)
                        lhs = w2[tp][k][:, m * 128 : (m + 1) * 128]
                        if PAIR_MM:
                            nc.tensor.matmul(
                                ps2[:],
                                lhs,
                                pv[:, :, ky : min(ky + 28, 29) : 2, 1 + kx : 29 + kx : 2],
                                start=first,
                                stop=(k == 1 and tp == 8),
                            )
                        else:
                            for i in (0, 1):
                                nc.tensor.matmul(
                                    ps2[:, i * 196 : (i + 1) * 196],
                                    lhs,
                                    pv[:, i, ky : min(ky + 28, 29) : 2, 1 + kx : 29 + kx : 2],
                                    start=first,
                                    stop=(k == 1 and tp == 8),
                                    skip_group_check=True,
                                )
                        first = False
                r2 = rp.tile([128, 392], F32, tag="r2")
                nc.scalar.activation(
                    r2[:], ps2[:], AFT.Relu, bias=beta2[:, m : m + 1], scale=a2
                )
                t3m = t3p.tile([128, 392], BF16, tag=f"t3_{m}")
                nc.vector.tensor_scalar(
                    t3m[:], r2[:], 127.0, 128.0, op0=ALU.min, op1=ALU.add
                )
                t3.append(t3m)
                if debug_taps and p == 0:
                    nc.sync.dma_start(t3_d[m], t3m[:])

            if debug_taps and p == 0:
                for m in range(2):
                    nc.sync.dma_start(p2_d[m], p2[m][:])

            # ---- conv3 (1x1, 256->1024) + shortcut (1x1 s2, 512->1024) ----
            for m in range(8):
                ps3 = pc3.tile([128, 392], F32)
                for k in range(2):
                    nc.tensor.matmul(
                        ps3[:],
                        w3[k][:, m * 128 : (m + 1) * 128],
                        t3[k][:],
                        start=(k == 0),
                        stop=False,
                        skip_group_check=True,
                    )
                for k in range(4):
                    xv = xq[k].rearrange("q (i r c) -> q i r c", i=2, r=28, c=28)
                    if PAIR_MM:
                        nc.tensor.matmul(
                            ps3[:],
                            ws[k][:, m * 128 : (m + 1) * 128],
                            xv[:, :, 0:28:2, 0:28:2],
                            start=False,
                            stop=(k == 3),
                            skip_group_check=True,
                        )
                    else:
                        for i in (0, 1):
                            nc.tensor.matmul(
                                ps3[:, i * 196 : (i + 1) * 196],
                                ws[k][:, m * 128 : (m + 1) * 128],
                                xv[:, i, 0:28:2, 0:28:2],
                                start=False,
                                stop=(k == 3),
                                skip_group_check=True,
                            )
                r3 = rp.tile([128, 392], F32, tag="r3")
                nc.scalar.activation(
                    r3[:], ps3[:], AFT.Relu, bias=delta[:, m : m + 1], scale=g3
                )
                yo = yop.tile([128, 392], F32)
                nc.vector.tensor_scalar(yo[:], r3[:], 6.0, None, op0=ALU.min)
                nc.sync.dma_start(
                    y_d[2 * p : 2 * p + 2, m].rearrange("i q h -> q i h"),
                    yo[:].rearrange("q (i h) -> q i h", i=2),
                )
    return nc


_SCALES = (1.0, 1.0, 1.0)


def _prep(w1, b1, w2, b2, w3, b3, wsw, bs):
    """Host-side weight quantization + constant folding (all tiny tensors)."""
    f32 = np.float32

    def qw(w):
        s = f32(np.max(np.abs(w)))
        wq = np.round(np.clip(w / s, f32(-1.0), f32(1.0)) * f32(127.0)).astype(
            np.float32
        )
        return wq, s

    def qb(b):
        return np.round(b * f32(127.0)).astype(np.float32)

    w1q, c1s = qw(w1)  # [256,512,1,1]
    w2q, c2s = qw(w2)  # [256,256,3,3]
    w3q, c3s = qw(w3)  # [1024,256,1,1]
    wsq, css = qw(wsw)  # [1024,512,1,1]
    B1, B2, B3, Bs = qb(b1), qb(b2), qb(b3), qb(bs)

    a1 = f32(2.0) * c1s / f32(127.0)
    a2 = f32(2.0) * c2s / f32(127.0)
    g3 = c3s / f32(2.0 * 16129.0)
    rho = css / c3s

    # lhsT layouts
    w1l = np.ascontiguousarray(
        w1q[:, :, 0, 0].T.reshape(4, 128, 256).astype(BF16NP)
    )
    # w2 taps: [ky,kx] -> lhsT [cin, cout] per tap
    w2l = np.ascontiguousarray(
        w2q.transpose(2, 3, 1, 0).reshape(9, 2, 128, 256).astype(BF16NP)
    )
    w3l = np.ascontiguousarray(
        w3q[:, :, 0, 0].T.reshape(2, 128, 1024).astype(BF16NP)
    )
    ws_sc = (rho * wsq[:, :, 0, 0]).astype(BF16NP)  # [1024,512] scaled bf16
    wsl = np.ascontiguousarray(ws_sc.T.reshape(4, 128, 1024))

    # column sums for the +128 activation offset corrections (fp64 exact)
    K1 = w1q[:, :, 0, 0].astype(np.float64).sum(axis=1)  # [256]
    K2 = w2q.astype(np.float64).sum(axis=(1, 2, 3))  # [256]
    K3 = w3q[:, :, 0, 0].astype(np.float64).sum(axis=1)  # [1024]
    Ks = ws_sc.astype(np.float64).sum(axis=1)  # [1024]

    beta1 = (f32(4.0) * B1 - a1 * f32(128.0) * K1.astype(np.float32)).astype(
        np.float32
    )
    beta2 = (f32(4.0) * B2 - a2 * f32(128.0) * K2.astype(np.float32)).astype(
        np.float32
    )
    delta0 = B3 * c3s / (f32(127.0) * c2s) + Bs / f32(127.0)
    delta = (
        delta0 - g3 * f32(128.0) * (K3 + Ks).astype(np.float32)
    ).astype(np.float32)

    beta1 = np.ascontiguousarray(beta1.reshape(2, 128).T)  # [128,2]
    beta2 = np.ascontiguousarray(beta2.reshape(2, 128).T)
    delta = np.ascontiguousarray(delta.reshape(8, 128).T)  # [128,8]

    return dict(
        w1l=w1l, w2l=w2l, w3l=w3l, wsl=wsl,
        beta1=beta1, beta2=beta2, delta=delta,
        a1=float(a1), a2=float(a2), g3=float(g3),
    )


def kernel(x, w1, b1, w2, b2, w3, b3, ws, bs):
    global _SCALES
    x = np.asarray(x, dtype=np.float32)
    pre = _prep(
        np.asarray(w1, np.float32), np.asarray(b1, np.float32),
        np.asarray(w2, np.float32), np.asarray(b2, np.float32),
        np.asarray(w3, np.float32), np.asarray(b3, np.float32),
        np.asarray(ws, np.float32), np.asarray(bs, np.float32),
    )
    _SCALES = (pre["a1"], pre["a2"], pre["g3"])
    nc = _build_nc()
    nc.compile()

    shared = {
        "w1l": pre["w1l"], "w2l": pre["w2l"], "w3l": pre["w3l"],
        "wsl": pre["wsl"], "beta1": pre["beta1"], "beta2": pre["beta2"],
        "delta": pre["delta"],
    }
    in_maps = []
    for c in range(N_CORES):
        xs = np.ascontiguousarray(
            x[c * B_LOC : (c + 1) * B_LOC].reshape(B_LOC, 4, 128, 784)
        )
        in_maps.append({"x": xs, **shared})

    import os

    tmpdir = os.environ.get("KERNEL_TRACE_DIR") or None
    if tmpdir:
        os.makedirs(tmpdir, exist_ok=True)
    res = run_bass_kernel_spmd(nc, in_maps, list(range(N_CORES)), tmpdir=tmpdir)
    global LAST_RESULT
    LAST_RESULT = res
    outs = [res.results[c]["y"] for c in range(N_CORES)]
    y = np.concatenate(outs, axis=0)  # [64, 8, 128, 196]
    return np.ascontiguousarray(y.reshape(64, 1024, 14, 14))


# revision 17
# speedup vs baseline: 3.5492x; 3.5492x over previous
"""Trainium2 Bass kernel for the quantized ResNet Bottleneck block.

Strategy
--------
Data parallel over batch: 64 images -> 8 cores x 8 images. Each core runs an
identical Bass program; weights are replicated.

All convs are executed as integer-valued bf16 matmuls accumulated in fp32
PSUM (exact: quantized codes are integers in [-127,127]; we add +128 so
activations live in [1,255], still exact bf16; the +128 offset passes through
each conv as a per-output-channel constant which is folded into the next
bias on the host).

Quantization rounding is done with two exact fp32 tricks:
  * x-quant: magic-constant add (C = 1.5*2^23) -> RNE round-to-integer.
  * conv epilogues: ReLU clamp on ACT, then (min 127) + 128 and convert
    fp32->bf16; in the [128,256) binade bf16 RNE == round-half-even.

conv3 (1x1) and the stride-2 shortcut conv accumulate into the same PSUM
tile; shortcut weights are pre-scaled by (css/c3s) on the host so both
contributions share one output scale.
"""

import sys
from contextlib import ExitStack

import numpy as np

sys.path.insert(0, "/opt/trn_rl_repo")

import ml_dtypes  # noqa: E402

import concourse.bacc as bacc  # noqa: E402
import concourse.bass as bass  # noqa: E402
import concourse.tile as tile  # noqa: E402
from concourse import mybir  # noqa: E402
from concourse.bass_utils import run_bass_kernel_spmd  # noqa: E402

F32 = mybir.dt.float32
BF16 = mybir.dt.bfloat16
ALU = mybir.AluOpType
AFT = mybir.ActivationFunctionType
BF16NP = ml_dtypes.bfloat16

C_MAGIC = float(np.float32(12582912.0))  # 1.5 * 2**23

N_CORES = 8
B_LOC = 8  # images per core
PAIRS = B_LOC // 2

# Use one matmul per image pair (4D rhs access pattern) where possible.
PAIR_MM = True


def _build_nc(pairs=PAIRS, debug_taps=False):
    nc = bacc.Bacc("TRN2", target_bir_lowering=False, debug=False)
    b_loc = 2 * pairs

    x_d = nc.dram_tensor("x", [b_loc, 4, 128, 784], F32, kind="ExternalInput")
    w1_d = nc.dram_tensor("w1l", [4, 128, 256], BF16, kind="ExternalInput")
    w2_d = nc.dram_tensor("w2l", [9, 2, 128, 256], BF16, kind="ExternalInput")
    w3_d = nc.dram_tensor("w3l", [2, 128, 1024], BF16, kind="ExternalInput")
    ws_d = nc.dram_tensor("wsl", [4, 128, 1024], BF16, kind="ExternalInput")
    b1_d = nc.dram_tensor("beta1", [128, 2], F32, kind="ExternalInput")
    b2_d = nc.dram_tensor("beta2", [128, 2], F32, kind="ExternalInput")
    dl_d = nc.dram_tensor("delta", [128, 8], F32, kind="ExternalInput")
    y_d = nc.dram_tensor("y", [b_loc, 8, 128, 196], F32, kind="ExternalOutput")
    if debug_taps:
        xq_d = nc.dram_tensor("dbg_xq", [4, 128, 1568], BF16, kind="ExternalOutput")
        p2_d = nc.dram_tensor("dbg_p2", [2, 128, 1856], BF16, kind="ExternalOutput")
        t3_d = nc.dram_tensor("dbg_t3", [2, 128, 392], BF16, kind="ExternalOutput")

    # scales (filled per-call via globals set by kernel(); see _SCALES)
    a1, a2, g3 = _SCALES

    with tile.TileContext(nc) as tc, ExitStack() as ctx:
        wp = ctx.enter_context(tc.tile_pool(name="w", bufs=1))
        xinp = ctx.enter_context(tc.tile_pool(name="xin", bufs=3))
        yap = ctx.enter_context(tc.tile_pool(name="ya", bufs=3))
        tbp = ctx.enter_context(tc.tile_pool(name="tb", bufs=3))
        xqp = ctx.enter_context(tc.tile_pool(name="xq", bufs=2))
        p2p = ctx.enter_context(tc.tile_pool(name="p2", bufs=2))
        t3p = ctx.enter_context(tc.tile_pool(name="t3", bufs=2))
        rp = ctx.enter_context(tc.tile_pool(name="r", bufs=4))
        yop = ctx.enter_context(tc.tile_pool(name="yo", bufs=4))
        pc1 = ctx.enter_context(tc.tile_pool(name="pc1", bufs=4, space="PSUM"))
        pc2 = ctx.enter_context(tc.tile_pool(name="pc2", bufs=2, space="PSUM"))
        pc3 = ctx.enter_context(tc.tile_pool(name="pc3", bufs=2, space="PSUM"))

        # ---- weights + biases (loaded once) ----
        w1 = []
        for k in range(4):
            t = wp.tile([128, 256], BF16, tag=f"w1_{k}")
            nc.sync.dma_start(t[:], w1_d[k])
            w1.append(t)
        w2 = []
        for tp in range(9):
            row = []
            for k in range(2):
                t = wp.tile([128, 256], BF16, tag=f"w2_{tp}_{k}")
                nc.sync.dma_start(t[:], w2_d[tp, k])
                row.append(t)
            w2.append(row)
        w3 = []
        for k in range(2):
            t = wp.tile([128, 1024], BF16, tag=f"w3_{k}")
            nc.sync.dma_start(t[:], w3_d[k])
            w3.append(t)
        ws = []
        for k in range(4):
            t = wp.tile([128, 1024], BF16, tag=f"ws_{k}")
            nc.sync.dma_start(t[:], ws_d[k])
            ws.append(t)
        beta1 = wp.tile([128, 2], F32, tag="beta1")
        nc.sync.dma_start(beta1[:], b1_d[:])
        beta2 = wp.tile([128, 2], F32, tag="beta2")
        nc.sync.dma_start(beta2[:], b2_d[:])
        delta = wp.tile([128, 8], F32, tag="delta")
        nc.sync.dma_start(delta[:], dl_d[:])
        cmagic = wp.tile([128, 1], F32, tag="cmagic")
        nc.vector.memset(cmagic[:], C_MAGIC)

        for p in range(pairs):
            # ---- load x pair + quantize: X' = round(clip(2x,-1,1)*127)+128 ----
            xq = []
            for k in range(4):
                xin = xinp.tile([128, 1568], F32)
                for i in (0, 1):
                    nc.sync.dma_start(
                        xin[:, i * 784 : (i + 1) * 784], x_d[2 * p + i, k]
                    )
                xqk = xqp.tile([128, 1568], BF16, tag=f"xq{k}")
                if k < 3:
                    # ACT-round path: u = C + round(254*x)
                    ya = yap.tile([128, 1568], F32)
                    nc.scalar.activation(
                        ya[:], xin[:], AFT.Identity, bias=cmagic[:], scale=254.0
                    )
                    tb = tbp.tile([128, 1568], F32)
                    nc.vector.tensor_scalar(
                        tb[:], ya[:], C_MAGIC - 127.0, C_MAGIC - 128.0,
                        op0=ALU.max, op1=ALU.subtract,
                    )
                    nc.vector.tensor_scalar(
                        xqk[:], tb[:], 255.0, None, op0=ALU.min
                    )
                else:
                    # DVE-round path (magic-add rounds in the fp32 ALU)
                    ya = yap.tile([128, 1568], F32)
                    nc.vector.tensor_scalar(
                        ya[:], xin[:], 0.5, 254.0, op0=ALU.min, op1=ALU.mult
                    )
                    tb = tbp.tile([128, 1568], F32)
                    nc.vector.tensor_scalar(
                        tb[:], ya[:], C_MAGIC, C_MAGIC - 127.0,
                        op0=ALU.add, op1=ALU.max,
                    )
                    nc.vector.tensor_scalar(
                        xqk[:], tb[:], C_MAGIC - 128.0, None, op0=ALU.subtract
                    )
                xq.append(xqk)
                if debug_taps and p == 0:
                    nc.sync.dma_start(xq_d[k], xqk[:])

            # ---- conv1 (1x1, 512->256) + epilogue into padded tile ----
            p2 = []
            for m in range(2):
                # pad value 128 == quantized zero in the +128-shifted domain
                t = p2p.tile([128, 1856], BF16, tag=f"p2_{m}")
                nc.vector.memset(t[:], 128.0)
                p2.append(t)
            for m in range(2):
                pv = p2[m].rearrange("q (i r c) -> q i r c", i=2, r=29, c=32)
                for i in (0, 1):
                    for hf in (0, 1):
                        ps = pc1.tile([128, 392], F32)
                        off = i * 784 + hf * 392
                        for k in range(4):
                            nc.tensor.matmul(
                                ps[:],
                                w1[k][:, m * 128 : (m + 1) * 128],
                                xq[k][:, off : off + 392],
                                start=(k == 0),
                                stop=(k == 3),
                            )
                        r = rp.tile([128, 392], F32, tag="r1")
                        nc.scalar.activation(
                            r[:], ps[:], AFT.Relu, bias=beta1[:, m : m + 1], scale=a1
                        )
                        nc.vector.tensor_scalar(
                            pv[:, i, 1 + 14 * hf : 15 + 14 * hf, 2:30],
                            r[:].rearrange("q (a b) -> q a b", a=14),
                            127.0,
                            128.0,
                            op0=ALU.min,
                            op1=ALU.add,
                        )

            # ---- conv2 (3x3 s2, 256->256) + epilogue ----
            t3 = []
            for m in range(2):
                ps2 = pc2.tile([128, 392], F32)
                first = True
                for k in range(2):
                    pv = p2[k].rearrange("q (i r c) -> q i r c", i=2, r=29, c=32)
                    for tp in range(9):
                        ky, kx = divmod(tp, 3)
                        lhs = w2[tp][k][:, m * 128 : (m + 1) * 128]
                        if PAIR_MM:
                            nc.tensor.matmul(
                                ps2[:],
                                lhs,
                                pv[:, :, ky : min(ky + 28, 29) : 2, 1 + kx : 29 + kx : 2],
                                start=first,
                                stop=(k == 1 and tp == 8),
                            )
                        else:
                            for i in (0, 1):
                                nc.tensor.matmul(
                                    ps2[:, i * 196 : (i + 1) * 196],
                                    lhs,
                                    pv[:, i, ky : min(ky + 28, 29) : 2, 1 + kx : 29 + kx : 2],
                                    start=first,
                                    stop=(k == 1 and tp == 8),
                                    skip_group_check=True,
                                )
                        first = False
                r2 = rp.tile([128, 392], F32, tag="r2")
                nc.scalar.activation(
                    r2[:], ps2[:], AFT.Relu, bias=beta2[:, m : m + 1], scale=a2
                )
                t3m = t3p.tile([128, 392], BF16, tag=f"t3_{m}")
                nc.vector.tensor_scalar(
                    t3m[:], r2[:], 127.0, 128.0, op0=ALU.min, op1=ALU.add
                )
                t3.append(t3m)
                if debug_taps and p == 0:
                    nc.sync.dma_start(t3_d[m], t3m[:])

            if debug_taps and p == 0:
                for m in range(2):
                    nc.sync.dma_start(p2_d[m], p2[m][:])

            # ---- conv3 (1x1, 256->1024) + shortcut (1x1 s2, 512->1024) ----
            for m in range(8):
                ps3 = pc3.tile([128, 392], F32)
                for k in range(2):
                    nc.tensor.matmul(
                        ps3[:],
                        w3[k][:, m * 128 : (m + 1) * 128],
                        t3[k][:],
                        start=(k == 0),
                        stop=False,
                        skip_group_check=True,
                    )
                for k in range(4):
                    xv = xq[k].rearrange("q (i r c) -> q i r c", i=2, r=28, c=28)
                    if PAIR_MM:
                        nc.tensor.matmul(
                            ps3[:],
                            ws[k][:, m * 128 : (m + 1) * 128],
                            xv[:, :, 0:28:2, 0:28:2],
                            start=False,
                            stop=(k == 3),
                            skip_group_check=True,
                        )
                    else:
                        for i in (0, 1):
                            nc.tensor.matmul(
                                ps3[:, i * 196 : (i + 1) * 196],
                                ws[k][:, m * 128 : (m + 1) * 128],
                                xv[:, i, 0:28:2, 0:28:2],
                                start=False,
                                stop=(k == 3),
                                skip_group_check=True,
                            )
                r3 = rp.tile([128, 392], F32, tag="r3")
                nc.scalar.activation(
                    r3[:], ps3[:], AFT.Relu, bias=delta[:, m : m + 1], scale=g3
                )
                yo = yop.tile([128, 392], F32)
                nc.vector.tensor_scalar(yo[:], r3[:], 6.0, None, op0=ALU.min)
                nc.sync.dma_start(
                    y_d[2 * p : 2 * p + 2, m].rearrange("i q h -> q i h"),
                    yo[:].rearrange("q (i h) -> q i h", i=2),
                )
    return nc


_SCALES = (1.0, 1.0, 1.0)


def _prep(w1, b1, w2, b2, w3, b3, wsw, bs):
    """Host-side weight quantization + constant folding (all tiny tensors)."""
    f32 = np.float32

    def qw(w):
        s = f32(np.max(np.abs(w)))
        wq = np.round(np.clip(w / s, f32(-1.0), f32(1.0)) * f32(127.0)).astype(
            np.float32
        )
        return wq, s

    def qb(b):
        return np.round(b * f32(127.0)).astype(np.float32)

    w1q, c1s = qw(w1)  # [256,512,1,1]
    w2q, c2s = qw(w2)  # [256,256,3,3]
    w3q, c3s = qw(w3)  # [1024,256,1,1]
    wsq, css = qw(wsw)  # [1024,512,1,1]
    B1, B2, B3, Bs = qb(b1), qb(b2), qb(b3), qb(bs)

    a1 = f32(2.0) * c1s / f32(127.0)
    a2 = f32(2.0) * c2s / f32(127.0)
    g3 = c3s / f32(2.0 * 16129.0)
    rho = css / c3s

    # lhsT layouts
    w1l = np.ascontiguousarray(
        w1q[:, :, 0, 0].T.reshape(4, 128, 256).astype(BF16NP)
    )
    # w2 taps: [ky,kx] -> lhsT [cin, cout] per tap
    w2l = np.ascontiguousarray(
        w2q.transpose(2, 3, 1, 0).reshape(9, 2, 128, 256).astype(BF16NP)
    )
    w3l = np.ascontiguousarray(
        w3q[:, :, 0, 0].T.reshape(2, 128, 1024).astype(BF16NP)
    )
    ws_sc = (rho * wsq[:, :, 0, 0]).astype(BF16NP)  # [1024,512] scaled bf16
    wsl = np.ascontiguousarray(ws_sc.T.reshape(4, 128, 1024))

    # column sums for the +128 activation offset corrections (fp64 exact)
    K1 = w1q[:, :, 0, 0].astype(np.float64).sum(axis=1)  # [256]
    K2 = w2q.astype(np.float64).sum(axis=(1, 2, 3))  # [256]
    K3 = w3q[:, :, 0, 0].astype(np.float64).sum(axis=1)  # [1024]
    Ks = ws_sc.astype(np.float64).sum(axis=1)  # [1024]

    beta1 = (f32(4.0) * B1 - a1 * f32(128.0) * K1.astype(np.float32)).astype(
        np.float32
    )
    beta2 = (f32(4.0) * B2 - a2 * f32(128.0) * K2.astype(np.float32)).astype(
        np.float32
    )
    delta0 = B3 * c3s / (f32(127.0) * c2s) + Bs / f32(127.0)
    delta = (
        delta0 - g3 * f32(128.0) * (K3 + Ks).astype(np.float32)
    ).astype(np.float32)

    beta1 = np.ascontiguousarray(beta1.reshape(2, 128).T)  # [128,2]
    beta2 = np.ascontiguousarray(beta2.reshape(2, 128).T)
    delta = np.ascontiguousarray(delta.reshape(8, 128).T)  # [128,8]

    return dict(
        w1l=w1l, w2l=w2l, w3l=w3l, wsl=wsl,
        beta1=beta1, beta2=beta2, delta=delta,
        a1=float(a1), a2=float(a2), g3=float(g3),
    )


def kernel(x, w1, b1, w2, b2, w3, b3, ws, bs):
    global _SCALES
    x = np.asarray(x, dtype=np.float32)
    pre = _prep(
        np.asarray(w1, np.float32), np.asarray(b1, np.float32),
        np.asarray(w2, np.float32), np.asarray(b2, np.float32),
        np.asarray(w3, np.float32), np.asarray(b3, np.float32),
        np.asarray(ws, np.float32), np.asarray(bs, np.float32),
    )
    _SCALES = (pre["a1"], pre["a2"], pre["g3"])
    nc = _build_nc()
    nc.compile()

    shared = {
        "w1l": pre["w1l"], "w2l": pre["w2l"], "w3l": pre["w3l"],
        "wsl": pre["wsl"], "beta1": pre["beta1"], "beta2": pre["beta2"],
        "delta": pre["delta"],
    }
    in_maps = []
    for c in range(N_CORES):
        xs = np.ascontiguousarray(
            x[c * B_LOC : (c + 1) * B_LOC].reshape(B_LOC, 4, 128, 784)
        )
        in_maps.append({"x": xs, **shared})

    import os

    tmpdir = os.environ.get("KERNEL_TRACE_DIR") or None
    if tmpdir:
        os.makedirs(tmpdir, exist_ok=True)
    res = run_bass_kernel_spmd(nc, in_maps, list(range(N_CORES)), tmpdir=tmpdir)
    global LAST_RESULT
    LAST_RESULT = res
    outs = [res.results[c]["y"] for c in range(N_CORES)]
    y = np.concatenate(outs, axis=0)  # [64, 8, 128, 196]
    return np.ascontiguousarray(y.reshape(64, 1024, 14, 14))


# revision 33
# speedup vs baseline: 3.5754x; 1.0074x over previous
"""Trainium2 Bass kernel for the quantized ResNet Bottleneck block.

Strategy
--------
Data parallel over batch: 64 images -> 8 cores x 8 images. Each core runs an
identical Bass program; weights are replicated.

All convs are executed as integer-valued bf16 matmuls accumulated in fp32
PSUM (exact: quantized codes are integers in [-127,127]; we add +128 so
activations live in [1,255], still exact bf16; the +128 offset passes through
each conv as a per-output-channel constant which is folded into the next
bias on the host).

Quantization rounding is done with two exact fp32 tricks:
  * x-quant: magic-constant add (C = 1.5*2^23) -> RNE round-to-integer.
  * conv epilogues: ReLU clamp on ACT, then (min 127) + 128 and convert
    fp32->bf16; in the [128,256) binade bf16 RNE == round-half-even.

conv3 (1x1) and the stride-2 shortcut conv accumulate into the same PSUM
tile; shortcut weights are pre-scaled by (css/c3s) on the host so both
contributions share one output scale.
"""

import sys
from contextlib import ExitStack

import numpy as np

sys.path.insert(0, "/opt/trn_rl_repo")

import ml_dtypes  # noqa: E402

import concourse.bacc as bacc  # noqa: E402
import concourse.bass as bass  # noqa: E402
import concourse.dve_ops as dve_ops  # noqa: E402
import concourse.tile as tile  # noqa: E402
from concourse import mybir  # noqa: E402
from concourse.bass_utils import run_bass_kernel_spmd  # noqa: E402
from concourse.dve_spec import (  # noqa: E402
    C0 as DC0,
    C1 as DC1,
    C2 as DC2,
    One as DOne,
    Spec,
    Src0 as DSrc0,
    Src1 as DSrc1,
    _has_src1,
    lower as dve_lower,
    maxx,
    minn,
    relu as drelu,
)
from concourse.dve_uop import DveOpSpec  # noqa: E402
from concourse.dve_table_gen import dve_ver_for  # noqa: E402
from concourse.dve_ops import DveOp  # noqa: E402

F32 = mybir.dt.float32
BF16 = mybir.dt.bfloat16
ALU = mybir.AluOpType
AFT = mybir.ActivationFunctionType
BF16NP = ml_dtypes.bfloat16

C_MAGIC = float(np.float32(12582912.0))  # 1.5 * 2**23

N_CORES = 8
B_LOC = 8  # images per core
PAIRS = B_LOC // 2

# Use one matmul per image pair (4D rhs access pattern) where possible.
PAIR_MM = True


def _register_dve_op(name, spec, subdim=False):
    """Register a custom DVE op at runtime (table is generated per-NEFF)."""
    for o in dve_ops.OPS:
        if o.name == name:
            return o
    row = dve_ops._CUSTOM_DVE_ROW_BASE + len(dve_ops.OPS)
    assert row < 0x20
    shas = {}
    for ver in ("v3", "v4"):
        tmp = DveOpSpec(
            name=name, opcode=row, uops=dve_lower(spec, ver=ver),
            rd1_en=_has_src1(spec),
        )
        shas[ver] = tmp.sha(ver)
    op = DveOp(name, spec, subdim=subdim, uops_sha=shas)
    dve_ops.OPS.append(op)
    dve_ops._SUB_OPCODE_FOR_NAME[name] = row
    dve_ops.CUSTOM_DVE_SPECS[name] = spec
    return op


# x-quant finisher: in0 = C + 127 + round(254*x) (from ACT magic-add).
# out = min(relu(in0 - C), 254) = clip(round(254x), -127, 127) + 127
XFIN = _register_dve_op(
    "BNECK_XFIN_ANT",
    Spec(
        body=minn(drelu(DSrc0 - DC0), DC1),
        reference=lambda in0, in1, s0, s1, imm2: np.minimum(
            np.maximum(in0 - s0, 0.0), s1
        ),
    ),
)

# quantize epilogue: v = in0*alpha + beta ; out = clip(round(v),0,127) + 128
# round via magic-add: u = (v + C) rounds to integer grid; clip in shifted
# domain [C, C+127]; subtract C-128.
def _b(in0, in1):
    """Sim helper: in1 streams elementwise on HW; align shapes for numpy."""
    if isinstance(in1, np.ndarray) and in1.size == in0.size:
        return in1.reshape(in0.shape)
    return in1


QEPI = _register_dve_op(
    "BNECK_QEPI_ANT",
    Spec(
        body=(minn(maxx((DSrc0 * DC0 + DSrc1) + DC1, DC1), DC1 + DC2) - DC1)
        + (DC2 + DOne),
        reference=lambda in0, in1, s0, s1, imm2: np.minimum(
            np.maximum(np.round(in0 * s0 + _b(in0, in1)), 0.0), imm2
        )
        + (imm2 + 1.0),
    ),
)

# final epilogue: out = min(relu(in0*gamma + delta), 6)
FEPI = _register_dve_op(
    "BNECK_FEPI_ANT",
    Spec(
        body=minn(drelu(DSrc0 * DC0 + DSrc1), DC1),
        reference=lambda in0, in1, s0, s1, imm2: np.minimum(
            np.maximum(in0 * s0 + _b(in0, in1), 0.0), s1
        ),
    ),
)


def _build_nc(pairs=PAIRS, debug_taps=False):
    nc = bacc.Bacc("TRN2", target_bir_lowering=False, debug=False)
    b_loc = 2 * pairs

    x_d = nc.dram_tensor("x", [b_loc, 4, 128, 784], F32, kind="ExternalInput")
    w1_d = nc.dram_tensor("w1l", [4, 128, 256], BF16, kind="ExternalInput")
    w2_d = nc.dram_tensor("w2l", [9, 2, 128, 256], BF16, kind="ExternalInput")
    w3_d = nc.dram_tensor("w3l", [2, 128, 1024], BF16, kind="ExternalInput")
    ws_d = nc.dram_tensor("wsl", [4, 128, 1024], BF16, kind="ExternalInput")
    b1_d = nc.dram_tensor("beta1", [128, 2], F32, kind="ExternalInput")
    b2_d = nc.dram_tensor("beta2", [128, 2], F32, kind="ExternalInput")
    dl_d = nc.dram_tensor("delta", [128, 8], F32, kind="ExternalInput")
    y_d = nc.dram_tensor("y", [b_loc, 8, 128, 196], F32, kind="ExternalOutput")
    if debug_taps:
        xq_d = nc.dram_tensor("dbg_xq", [4, 128, 1568], BF16, kind="ExternalOutput")
        p2_d = nc.dram_tensor("dbg_p2", [2, 128, 1856], BF16, kind="ExternalOutput")
        t3_d = nc.dram_tensor("dbg_t3", [2, 128, 392], BF16, kind="ExternalOutput")

    # scales (filled per-call via globals set by kernel(); see _SCALES)
    a1, a2, g3 = _SCALES

    with tile.TileContext(nc) as tc, ExitStack() as ctx:
        wp = ctx.enter_context(tc.tile_pool(name="w", bufs=1))
        xinp = ctx.enter_context(tc.tile_pool(name="xin", bufs=3))
        yap = ctx.enter_context(tc.tile_pool(name="ya", bufs=3))
        xqp = ctx.enter_context(tc.tile_pool(name="xq", bufs=2))
        p2p = ctx.enter_context(tc.tile_pool(name="p2", bufs=2))
        t3p = ctx.enter_context(tc.tile_pool(name="t3", bufs=2))
        rp = ctx.enter_context(tc.tile_pool(name="r", bufs=4))
        yop = ctx.enter_context(tc.tile_pool(name="yo", bufs=4))
        pc1 = ctx.enter_context(tc.tile_pool(name="pc1", bufs=4, space="PSUM"))
        pc2 = ctx.enter_context(tc.tile_pool(name="pc2", bufs=2, space="PSUM"))
        pc3 = ctx.enter_context(tc.tile_pool(name="pc3", bufs=2, space="PSUM"))

        # ---- weights + biases (loaded once) ----
        w1 = []
        for k in range(4):
            t = wp.tile([128, 256], BF16, tag=f"w1_{k}")
            nc.sync.dma_start(t[:], w1_d[k])
            w1.append(t)
        w2 = []
        for tp in range(9):
            row = []
            for k in range(2):
                t = wp.tile([128, 256], BF16, tag=f"w2_{tp}_{k}")
                nc.sync.dma_start(t[:], w2_d[tp, k])
                row.append(t)
            w2.append(row)
        w3 = []
        for k in range(2):
            t = wp.tile([128, 1024], BF16, tag=f"w3_{k}")
            nc.sync.dma_start(t[:], w3_d[k])
            w3.append(t)
        ws = []
        for k in range(4):
            t = wp.tile([128, 1024], BF16, tag=f"ws_{k}")
            nc.sync.dma_start(t[:], ws_d[k])
            ws.append(t)
        beta1 = wp.tile([128, 2], F32, tag="beta1")
        nc.sync.dma_start(beta1[:], b1_d[:])
        beta2 = wp.tile([128, 2], F32, tag="beta2")
        nc.sync.dma_start(beta2[:], b2_d[:])
        delta = wp.tile([128, 8], F32, tag="delta")
        nc.sync.dma_start(delta[:], dl_d[:])
        c127 = wp.tile([128, 1], F32, tag="c127")
        nc.vector.memset(c127[:], C_MAGIC + 127.0)
        # broadcast bias rows: custom-DVE src1 must stream elementwise
        b1f, b2f = [], []
        for m in range(2):
            t = wp.tile([128, 392], F32, tag=f"b1f{m}")
            nc.vector.tensor_copy(t[:], beta1[:, m : m + 1].to_broadcast((128, 392)))
            b1f.append(t)
            t = wp.tile([128, 392], F32, tag=f"b2f{m}")
            nc.vector.tensor_copy(t[:], beta2[:, m : m + 1].to_broadcast((128, 392)))
            b2f.append(t)

        for p in range(pairs):
            # ---- load x pair + quantize: X' = round(clip(2x,-1,1)*127)+128 ----
            xq = []
            for k in range(4):
                xin = xinp.tile([128, 1568], F32)
                for i in (0, 1):
                    nc.sync.dma_start(
                        xin[:, i * 784 : (i + 1) * 784], x_d[2 * p + i, k]
                    )
                xqk = xqp.tile([128, 1568], BF16, tag=f"xq{k}")
                # u = C + 127 + round(254*x)  (ACT fused mul+add rounds at C)
                ya = yap.tile([128, 1568], F32)
                nc.scalar.activation(
                    ya[:], xin[:], AFT.Identity, bias=c127[:], scale=254.0
                )
                # X'' = min(relu(u - C), 254) = clip(round(254x),-127,127)+127
                nc.vector._custom_dve(
                    XFIN, out=xqk[:], in0=ya[:], s0=C_MAGIC, s1=254.0
                )
                xq.append(xqk)
                if debug_taps and p == 0:
                    nc.sync.dma_start(xq_d[k], xqk[:])

            # ---- conv1 (1x1, 512->256) + epilogue into padded tile ----
            p2 = []
            for m in range(2):
                # pad value 128 == quantized zero in the +128-shifted domain
                t = p2p.tile([128, 1856], BF16, tag=f"p2_{m}")
                nc.vector.memset(t[:], 128.0)
                p2.append(t)
            for m in range(2):
                pv = p2[m].rearrange("q (i r c) -> q i r c", i=2, r=29, c=32)
                for i in (0, 1):
                    for hf in (0, 1):
                        ps = pc1.tile([128, 392], F32)
                        off = i * 784 + hf * 392
                        for k in range(4):
                            nc.tensor.matmul(
                                ps[:],
                                w1[k][:, m * 128 : (m + 1) * 128],
                                xq[k][:, off : off + 392],
                                start=(k == 0),
                                stop=(k == 3),
                            )
                        nc.vector._custom_dve(
                            QEPI,
                            out=pv[:, i, 1 + 14 * hf : 15 + 14 * hf, 2:30],
                            in0=ps[:].rearrange("q (a b) -> q a b", a=14),
                            in1=b1f[m][:],
                            s0=a1,
                            s1=C_MAGIC,
                            imm2=127.0,
                        )

            # ---- conv2 (3x3 s2, 256->256) + epilogue ----
            t3 = []
            for m in range(2):
                ps2 = pc2.tile([128, 392], F32)
                first = True
                for k in range(2):
                    pv = p2[k].rearrange("q (i r c) -> q i r c", i=2, r=29, c=32)
                    for tp in range(9):
                        ky, kx = divmod(tp, 3)
                        lhs = w2[tp][k][:, m * 128 : (m + 1) * 128]
                        if PAIR_MM:
                            nc.tensor.matmul(
                                ps2[:],
                                lhs,
                                pv[:, :, ky : min(ky + 28, 29) : 2, 1 + kx : 29 + kx : 2],
                                start=first,
                                stop=(k == 1 and tp == 8),
                            )
                        else:
                            for i in (0, 1):
                                nc.tensor.matmul(
                                    ps2[:, i * 196 : (i + 1) * 196],
                                    lhs,
                                    pv[:, i, ky : min(ky + 28, 29) : 2, 1 + kx : 29 + kx : 2],
                                    start=first,
                                    stop=(k == 1 and tp == 8),
                                    skip_group_check=True,
                                )
                        first = False
                t3m = t3p.tile([128, 392], BF16, tag=f"t3_{m}")
                nc.vector._custom_dve(
                    QEPI,
                    out=t3m[:],
                    in0=ps2[:],
                    in1=b2f[m][:],
                    s0=a2,
                    s1=C_MAGIC,
                    imm2=127.0,
                )
                t3.append(t3m)
                if debug_taps and p == 0:
                    nc.sync.dma_start(t3_d[m], t3m[:])

            if debug_taps and p == 0:
                for m in range(2):
                    nc.sync.dma_start(p2_d[m], p2[m][:])

            # ---- conv3 (1x1, 256->1024) + shortcut (1x1 s2, 512->1024) ----
            for m in range(8):
                ps3 = pc3.tile([128, 392], F32)
                for k in range(2):
                    nc.tensor.matmul(
                        ps3[:],
                        w3[k][:, m * 128 : (m + 1) * 128],
                        t3[k][:],
                        start=(k == 0),
                        stop=False,
                        skip_group_check=True,
                    )
                for k in range(4):
                    xv = xq[k].rearrange("q (i r c) -> q i r c", i=2, r=28, c=28)
                    if PAIR_MM:
                        nc.tensor.matmul(
                            ps3[:],
                            ws[k][:, m * 128 : (m + 1) * 128],
                            xv[:, :, 0:28:2, 0:28:2],
                            start=False,
                            stop=(k == 3),
                            skip_group_check=True,
                        )
                    else:
                        for i in (0, 1):
                            nc.tensor.matmul(
                                ps3[:, i * 196 : (i + 1) * 196],
                                ws[k][:, m * 128 : (m + 1) * 128],
                                xv[:, i, 0:28:2, 0:28:2],
                                start=False,
                                stop=(k == 3),
                                skip_group_check=True,
                            )
                r3 = rp.tile([128, 392], F32, tag="r3")
                nc.scalar.activation(
                    r3[:], ps3[:], AFT.Relu, bias=delta[:, m : m + 1], scale=g3
                )
                yo = yop.tile([128, 392], F32)
                nc.vector.tensor_scalar(yo[:], r3[:], 6.0, None, op0=ALU.min)
                nc.sync.dma_start(
                    y_d[2 * p : 2 * p + 2, m].rearrange("i q h -> q i h"),
                    yo[:].rearrange("q (i h) -> q i h", i=2),
                )
    return nc


_SCALES = (1.0, 1.0, 1.0)


def _prep(w1, b1, w2, b2, w3, b3, wsw, bs):
    """Host-side weight quantization + constant folding (all tiny tensors)."""
    f32 = np.float32

    def qw(w):
        s = f32(np.max(np.abs(w)))
        wq = np.round(np.clip(w / s, f32(-1.0), f32(1.0)) * f32(127.0)).astype(
            np.float32
        )
        return wq, s

    def qb(b):
        return np.round(b * f32(127.0)).astype(np.float32)

    w1q, c1s = qw(w1)  # [256,512,1,1]
    w2q, c2s = qw(w2)  # [256,256,3,3]
    w3q, c3s = qw(w3)  # [1024,256,1,1]
    wsq, css = qw(wsw)  # [1024,512,1,1]
    B1, B2, B3, Bs = qb(b1), qb(b2), qb(b3), qb(bs)

    a1 = f32(2.0) * c1s / f32(127.0)
    a2 = f32(2.0) * c2s / f32(127.0)
    g3 = c3s / f32(2.0 * 16129.0)
    rho = css / c3s

    # lhsT layouts
    w1l = np.ascontiguousarray(
        w1q[:, :, 0, 0].T.reshape(4, 128, 256).astype(BF16NP)
    )
    # w2 taps: [ky,kx] -> lhsT [cin, cout] per tap
    w2l = np.ascontiguousarray(
        w2q.transpose(2, 3, 1, 0).reshape(9, 2, 128, 256).astype(BF16NP)
    )
    w3l = np.ascontiguousarray(
        w3q[:, :, 0, 0].T.reshape(2, 128, 1024).astype(BF16NP)
    )
    ws_sc = (rho * wsq[:, :, 0, 0]).astype(BF16NP)  # [1024,512] scaled bf16
    wsl = np.ascontiguousarray(ws_sc.T.reshape(4, 128, 1024))

    # column sums for the +128 activation offset corrections (fp64 exact)
    K1 = w1q[:, :, 0, 0].astype(np.float64).sum(axis=1)  # [256]
    K2 = w2q.astype(np.float64).sum(axis=(1, 2, 3))  # [256]
    K3 = w3q[:, :, 0, 0].astype(np.float64).sum(axis=1)  # [1024]
    Ks = ws_sc.astype(np.float64).sum(axis=1)  # [1024]

    # activation shifts: x-quant adds +127 (XFIN), conv epilogues add +128
    beta1 = (f32(4.0) * B1 - a1 * f32(127.0) * K1.astype(np.float32)).astype(
        np.float32
    )
    beta2 = (f32(4.0) * B2 - a2 * f32(128.0) * K2.astype(np.float32)).astype(
        np.float32
    )
    delta0 = B3 * c3s / (f32(127.0) * c2s) + Bs / f32(127.0)
    delta = (
        delta0
        - g3 * (f32(128.0) * K3 + f32(127.0) * Ks).astype(np.float32)
    ).astype(np.float32)

    beta1 = np.ascontiguousarray(beta1.reshape(2, 128).T)  # [128,2]
    beta2 = np.ascontiguousarray(beta2.reshape(2, 128).T)
    delta = np.ascontiguousarray(delta.reshape(8, 128).T)  # [128,8]

    return dict(
        w1l=w1l, w2l=w2l, w3l=w3l, wsl=wsl,
        beta1=beta1, beta2=beta2, delta=delta,
        a1=float(a1), a2=float(a2), g3=float(g3),
    )


def kernel(x, w1, b1, w2, b2, w3, b3, ws, bs):
    global _SCALES
    x = np.asarray(x, dtype=np.float32)
    pre = _prep(
        np.asarray(w1, np.float32), np.asarray(b1, np.float32),
        np.asarray(w2, np.float32), np.asarray(b2, np.float32),
        np.asarray(w3, np.float32), np.asarray(b3, np.float32),
        np.asarray(ws, np.float32), np.asarray(bs, np.float32),
    )
    _SCALES = (pre["a1"], pre["a2"], pre["g3"])
    nc = _build_nc()
    nc.compile()

    shared = {
        "w1l": pre["w1l"], "w2l": pre["w2l"], "w3l": pre["w3l"],
        "wsl": pre["wsl"], "beta1": pre["beta1"], "beta2": pre["beta2"],
        "delta": pre["delta"],
    }
    in_maps = []
    for c in range(N_CORES):
        xs = np.ascontiguousarray(
            x[c * B_LOC : (c + 1) * B_LOC].reshape(B_LOC, 4, 128, 784)
        )
        in_maps.append({"x": xs, **shared})

    import os

    tmpdir = os.environ.get("KERNEL_TRACE_DIR") or None
    if tmpdir:
        os.makedirs(tmpdir, exist_ok=True)
    res = run_bass_kernel_spmd(nc, in_maps, list(range(N_CORES)), tmpdir=tmpdir)
    global LAST_RESULT
    LAST_RESULT = res
    outs = [res.results[c]["y"] for c in range(N_CORES)]
    y = np.concatenate(outs, axis=0)  # [64, 8, 128, 196]
    return np.ascontiguousarray(y.reshape(64, 1024, 14, 14))


# revision 36
# speedup vs baseline: 3.8338x; 1.0723x over previous
"""Trainium2 Bass kernel for the quantized ResNet Bottleneck block.

Strategy
--------
Data parallel over batch: 64 images -> 8 cores x 8 images. Each core runs an
identical Bass program; weights are replicated.

All convs are executed as integer-valued bf16 matmuls accumulated in fp32
PSUM (exact: quantized codes are integers in [-127,127]; we add +128 so
activations live in [1,255], still exact bf16; the +128 offset passes through
each conv as a per-output-channel constant which is folded into the next
bias on the host).

Quantization rounding is done with two exact fp32 tricks:
  * x-quant: magic-constant add (C = 1.5*2^23) -> RNE round-to-integer.
  * conv epilogues: ReLU clamp on ACT, then (min 127) + 128 and convert
    fp32->bf16; in the [128,256) binade bf16 RNE == round-half-even.

conv3 (1x1) and the stride-2 shortcut conv accumulate into the same PSUM
tile; shortcut weights are pre-scaled by (css/c3s) on the host so both
contributions share one output scale.
"""

import sys
from contextlib import ExitStack

import numpy as np

sys.path.insert(0, "/opt/trn_rl_repo")

import ml_dtypes  # noqa: E402

import concourse.bacc as bacc  # noqa: E402
import concourse.bass as bass  # noqa: E402
import concourse.dve_ops as dve_ops  # noqa: E402
import concourse.tile as tile  # noqa: E402
from concourse import mybir  # noqa: E402
from concourse.bass_utils import run_bass_kernel_spmd  # noqa: E402
from concourse.dve_spec import (  # noqa: E402
    C0 as DC0,
    C1 as DC1,
    C2 as DC2,
    One as DOne,
    Spec,
    Src0 as DSrc0,
    Src1 as DSrc1,
    _has_src1,
    lower as dve_lower,
    maxx,
    minn,
    relu as drelu,
)
from concourse.dve_uop import DveOpSpec  # noqa: E402
from concourse.dve_table_gen import dve_ver_for  # noqa: E402
from concourse.dve_ops import DveOp  # noqa: E402

F32 = mybir.dt.float32
BF16 = mybir.dt.bfloat16
ALU = mybir.AluOpType
AFT = mybir.ActivationFunctionType
BF16NP = ml_dtypes.bfloat16

C_MAGIC = float(np.float32(12582912.0))  # 1.5 * 2**23

N_CORES = 8
B_LOC = 8  # images per core
PAIRS = B_LOC // 2

# Use one matmul per image pair (4D rhs access pattern) where possible.
PAIR_MM = True


def _register_dve_op(name, spec, subdim=False):
    """Register a custom DVE op at runtime (table is generated per-NEFF)."""
    for o in dve_ops.OPS:
        if o.name == name:
            return o
    row = dve_ops._CUSTOM_DVE_ROW_BASE + len(dve_ops.OPS)
    assert row < 0x20
    shas = {}
    for ver in ("v3", "v4"):
        tmp = DveOpSpec(
            name=name, opcode=row, uops=dve_lower(spec, ver=ver),
            rd1_en=_has_src1(spec),
        )
        shas[ver] = tmp.sha(ver)
    op = DveOp(name, spec, subdim=subdim, uops_sha=shas)
    dve_ops.OPS.append(op)
    dve_ops._SUB_OPCODE_FOR_NAME[name] = row
    dve_ops.CUSTOM_DVE_SPECS[name] = spec
    return op


# x-quant finisher: in0 = C + 127 + round(254*x) (from ACT magic-add).
# out = min(relu(in0 - C), 254) = clip(round(254x), -127, 127) + 127
XFIN = _register_dve_op(
    "BNECK_XFIN_ANT",
    Spec(
        body=minn(drelu(DSrc0 - DC0), DC1),
        reference=lambda in0, in1, s0, s1, imm2: np.minimum(
            np.maximum(in0 - s0, 0.0), s1
        ),
    ),
)

# quantize epilogue: v = in0*alpha + beta ; out = clip(round(v),0,127) + 128
# round via magic-add: u = (v + C) rounds to integer grid; clip in shifted
# domain [C, C+127]; subtract C-128.
def _b(in0, in1):
    """Sim helper: in1 streams elementwise on HW; align shapes for numpy."""
    if isinstance(in1, np.ndarray) and in1.size == in0.size:
        return in1.reshape(in0.shape)
    return in1


QEPI = _register_dve_op(
    "BNECK_QEPI_ANT",
    Spec(
        body=(minn(maxx((DSrc0 * DC0 + DSrc1) + DC1, DC1), DC1 + DC2) - DC1)
        + (DC2 + DOne),
        reference=lambda in0, in1, s0, s1, imm2: np.minimum(
            np.maximum(np.round(in0 * s0 + _b(in0, in1)), 0.0), imm2
        )
        + (imm2 + 1.0),
    ),
)

# final epilogue: out = min(relu(in0*gamma + delta), 6)
FEPI = _register_dve_op(
    "BNECK_FEPI_ANT",
    Spec(
        body=minn(drelu(DSrc0 * DC0 + DSrc1), DC1),
        reference=lambda in0, in1, s0, s1, imm2: np.minimum(
            np.maximum(in0 * s0 + _b(in0, in1), 0.0), s1
        ),
    ),
)


def _build_nc(pairs=PAIRS, debug_taps=False):
    nc = bacc.Bacc("TRN2", target_bir_lowering=False, debug=False)
    b_loc = 2 * pairs

    x_d = nc.dram_tensor("x", [b_loc, 4, 128, 784], F32, kind="ExternalInput")
    w1_d = nc.dram_tensor("w1l", [4, 128, 256], BF16, kind="ExternalInput")
    w2_d = nc.dram_tensor("w2l", [9, 2, 128, 256], BF16, kind="ExternalInput")
    w3_d = nc.dram_tensor("w3l", [2, 128, 1024], BF16, kind="ExternalInput")
    ws_d = nc.dram_tensor("wsl", [4, 128, 1024], BF16, kind="ExternalInput")
    b1_d = nc.dram_tensor("beta1", [128, 2], F32, kind="ExternalInput")
    b2_d = nc.dram_tensor("beta2", [128, 2], F32, kind="ExternalInput")
    dl_d = nc.dram_tensor("delta", [128, 8], F32, kind="ExternalInput")
    y_d = nc.dram_tensor("y", [b_loc, 8, 128, 196], F32, kind="ExternalOutput")
    if debug_taps:
        xq_d = nc.dram_tensor("dbg_xq", [4, 128, 1568], BF16, kind="ExternalOutput")
        p2_d = nc.dram_tensor("dbg_p2", [2, 128, 1856], BF16, kind="ExternalOutput")
        t3_d = nc.dram_tensor("dbg_t3", [2, 128, 392], BF16, kind="ExternalOutput")

    # scales (filled per-call via globals set by kernel(); see _SCALES)
    a1, a2, g3 = _SCALES

    with tile.TileContext(nc) as tc, ExitStack() as ctx:
        wp = ctx.enter_context(tc.tile_pool(name="w", bufs=1))
        xinp = ctx.enter_context(tc.tile_pool(name="xin", bufs=3))
        yap = ctx.enter_context(tc.tile_pool(name="ya", bufs=3))
        xqp = ctx.enter_context(tc.tile_pool(name="xq", bufs=2))
        p2p = ctx.enter_context(tc.tile_pool(name="p2", bufs=2))
        t3p = ctx.enter_context(tc.tile_pool(name="t3", bufs=2))
        rp = ctx.enter_context(tc.tile_pool(name="r", bufs=4))
        yop = ctx.enter_context(tc.tile_pool(name="yo", bufs=4))
        pc1 = ctx.enter_context(tc.tile_pool(name="pc1", bufs=4, space="PSUM"))
        pc2 = ctx.enter_context(tc.tile_pool(name="pc2", bufs=2, space="PSUM"))
        pc3 = ctx.enter_context(tc.tile_pool(name="pc3", bufs=2, space="PSUM"))

        # ---- weights + biases (loaded once) ----
        # w1 + conv1 biases load first so pair-0 conv1 can start ASAP
        w1 = []
        for k in range(4):
            t = wp.tile([128, 256], BF16, tag=f"w1_{k}")
            nc.sync.dma_start(t[:], w1_d[k])
            w1.append(t)
        beta1 = wp.tile([128, 2], F32, tag="beta1")
        nc.sync.dma_start(beta1[:], b1_d[:])
        c127 = wp.tile([128, 1], F32, tag="c127")
        nc.vector.memset(c127[:], C_MAGIC + 127.0)
        b1f, b2f = [], []
        for m in range(2):
            t = wp.tile([128, 392], F32, tag=f"b1f{m}")
            nc.vector.tensor_copy(t[:], beta1[:, m : m + 1].to_broadcast((128, 392)))
            b1f.append(t)

        st = {}  # per-pair tiles: xq, p2, t3

        def emit_xload(p):
            xq = []
            for k in range(4):
                xin = xinp.tile([128, 1568], F32)
                for i in (0, 1):
                    nc.sync.dma_start(
                        xin[:, i * 784 : (i + 1) * 784], x_d[2 * p + i, k]
                    )
                xqk = xqp.tile([128, 1568], BF16, tag=f"xq{k}")
                # u = C + 127 + round(254*x)  (ACT fused mul+add rounds at C)
                ya = yap.tile([128, 1568], F32)
                nc.scalar.activation(
                    ya[:], xin[:], AFT.Identity, bias=c127[:], scale=254.0
                )
                # X'' = min(relu(u - C), 254) = clip(round(254x),-127,127)+127
                nc.vector._custom_dve(
                    XFIN, out=xqk[:], in0=ya[:], s0=C_MAGIC, s1=254.0
                )
                xq.append(xqk)
                if debug_taps and p == 0:
                    nc.sync.dma_start(xq_d[k], xqk[:])
            st[p] = {"xq": xq}

        def emit_conv1(p):
            xq = st[p]["xq"]
            p2 = []
            for m in range(2):
                # pad value 128 == quantized zero in the +128-shifted domain
                t = p2p.tile([128, 1856], BF16, tag=f"p2_{m}")
                nc.vector.memset(t[:], 128.0)
                p2.append(t)
            for m in range(2):
                pv = p2[m].rearrange("q (i r c) -> q i r c", i=2, r=29, c=32)
                for i in (0, 1):
                    for hf in (0, 1):
                        ps = pc1.tile([128, 392], F32)
                        off = i * 784 + hf * 392
                        for k in range(4):
                            nc.tensor.matmul(
                                ps[:],
                                w1[k][:, m * 128 : (m + 1) * 128],
                                xq[k][:, off : off + 392],
                                start=(k == 0),
                                stop=(k == 3),
                            )
                        nc.vector._custom_dve(
                            QEPI,
                            out=pv[:, i, 1 + 14 * hf : 15 + 14 * hf, 2:30],
                            in0=ps[:].rearrange("q (a b) -> q a b", a=14),
                            in1=b1f[m][:],
                            s0=a1,
                            s1=C_MAGIC,
                            imm2=127.0,
                        )
            st[p]["p2"] = p2

        def emit_conv2(p):
            p2 = st[p]["p2"]
            t3 = []
            for m in range(2):
                ps2 = pc2.tile([128, 392], F32)
                first = True
                for k in range(2):
                    pv = p2[k].rearrange("q (i r c) -> q i r c", i=2, r=29, c=32)
                    for tp in range(9):
                        ky, kx = divmod(tp, 3)
                        nc.tensor.matmul(
                            ps2[:],
                            w2[tp][k][:, m * 128 : (m + 1) * 128],
                            pv[:, :, ky : min(ky + 28, 29) : 2, 1 + kx : 29 + kx : 2],
                            start=first,
                            stop=(k == 1 and tp == 8),
                        )
                        first = False
                t3m = t3p.tile([128, 392], BF16, tag=f"t3_{m}")
                nc.vector._custom_dve(
                    QEPI,
                    out=t3m[:],
                    in0=ps2[:],
                    in1=b2f[m][:],
                    s0=a2,
                    s1=C_MAGIC,
                    imm2=127.0,
                )
                t3.append(t3m)
                if debug_taps and p == 0:
                    nc.sync.dma_start(t3_d[m], t3m[:])
            if debug_taps and p == 0:
                for m in range(2):
                    nc.sync.dma_start(p2_d[m], p2[m][:])
            st[p]["t3"] = t3

        def emit_conv3(p):
            xq, t3 = st[p]["xq"], st[p]["t3"]
            for m in range(8):
                ps3 = pc3.tile([128, 392], F32)
                # shortcut first: only needs xq, giving t3's epilogue time
                for k in range(4):
                    xv = xq[k].rearrange("q (i r c) -> q i r c", i=2, r=28, c=28)
                    nc.tensor.matmul(
                        ps3[:],
                        ws[k][:, m * 128 : (m + 1) * 128],
                        xv[:, :, 0:28:2, 0:28:2],
                        start=(k == 0),
                        stop=False,
                        skip_group_check=True,
                    )
                for k in range(2):
                    nc.tensor.matmul(
                        ps3[:],
                        w3[k][:, m * 128 : (m + 1) * 128],
                        t3[k][:],
                        start=False,
                        stop=(k == 1),
                        skip_group_check=True,
                    )
                r3 = rp.tile([128, 392], F32, tag="r3")
                nc.scalar.activation(
                    r3[:], ps3[:], AFT.Relu, bias=delta[:, m : m + 1], scale=g3
                )
                yo = yop.tile([128, 392], F32)
                nc.vector.tensor_scalar(yo[:], r3[:], 6.0, None, op0=ALU.min)
                nc.sync.dma_start(
                    y_d[2 * p : 2 * p + 2, m].rearrange("i q h -> q i h"),
                    yo[:].rearrange("q (i h) -> q i h", i=2),
                )
            del st[p]

        # pair 0 front-end before the bulk weight loads
        emit_xload(0)

        w2 = []
        for tp in range(9):
            row = []
            for k in range(2):
                t = wp.tile([128, 256], BF16, tag=f"w2_{tp}_{k}")
                nc.sync.dma_start(t[:], w2_d[tp, k])
                row.append(t)
            w2.append(row)
        w3 = []
        for k in range(2):
            t = wp.tile([128, 1024], BF16, tag=f"w3_{k}")
            nc.sync.dma_start(t[:], w3_d[k])
            w3.append(t)
        ws = []
        for k in range(4):
            t = wp.tile([128, 1024], BF16, tag=f"ws_{k}")
            nc.sync.dma_start(t[:], ws_d[k])
            ws.append(t)
        beta2 = wp.tile([128, 2], F32, tag="beta2")
        nc.sync.dma_start(beta2[:], b2_d[:])
        delta = wp.tile([128, 8], F32, tag="delta")
        nc.sync.dma_start(delta[:], dl_d[:])
        for m in range(2):
            t = wp.tile([128, 392], F32, tag=f"b2f{m}")
            nc.vector.tensor_copy(t[:], beta2[:, m : m + 1].to_broadcast((128, 392)))
            b2f.append(t)

        # software pipeline: conv2/conv3 of pair p-1 run under conv1 of pair p
        emit_conv1(0)
        for p in range(1, pairs):
            emit_xload(p)
            emit_conv1(p)
            emit_conv2(p - 1)
            emit_conv3(p - 1)
        emit_conv2(pairs - 1)
        emit_conv3(pairs - 1)
    return nc


_SCALES = (1.0, 1.0, 1.0)


def _prep(w1, b1, w2, b2, w3, b3, wsw, bs):
    """Host-side weight quantization + constant folding (all tiny tensors)."""
    f32 = np.float32

    def qw(w):
        s = f32(np.max(np.abs(w)))
        wq = np.round(np.clip(w / s, f32(-1.0), f32(1.0)) * f32(127.0)).astype(
            np.float32
        )
        return wq, s

    def qb(b):
        return np.round(b * f32(127.0)).astype(np.float32)

    w1q, c1s = qw(w1)  # [256,512,1,1]
    w2q, c2s = qw(w2)  # [256,256,3,3]
    w3q, c3s = qw(w3)  # [1024,256,1,1]
    wsq, css = qw(wsw)  # [1024,512,1,1]
    B1, B2, B3, Bs = qb(b1), qb(b2), qb(b3), qb(bs)

    a1 = f32(2.0) * c1s / f32(127.0)
    a2 = f32(2.0) * c2s / f32(127.0)
    g3 = c3s / f32(2.0 * 16129.0)
    rho = css / c3s

    # lhsT layouts
    w1l = np.ascontiguousarray(
        w1q[:, :, 0, 0].T.reshape(4, 128, 256).astype(BF16NP)
    )
    # w2 taps: [ky,kx] -> lhsT [cin, cout] per tap
    w2l = np.ascontiguousarray(
        w2q.transpose(2, 3, 1, 0).reshape(9, 2, 128, 256).astype(BF16NP)
    )
    w3l = np.ascontiguousarray(
        w3q[:, :, 0, 0].T.reshape(2, 128, 1024).astype(BF16NP)
    )
    ws_sc = (rho * wsq[:, :, 0, 0]).astype(BF16NP)  # [1024,512] scaled bf16
    wsl = np.ascontiguousarray(ws_sc.T.reshape(4, 128, 1024))

    # column sums for the +128 activation offset corrections (fp64 exact)
    K1 = w1q[:, :, 0, 0].astype(np.float64).sum(axis=1)  # [256]
    K2 = w2q.astype(np.float64).sum(axis=(1, 2, 3))  # [256]
    K3 = w3q[:, :, 0, 0].astype(np.float64).sum(axis=1)  # [1024]
    Ks = ws_sc.astype(np.float64).sum(axis=1)  # [1024]

    # activation shifts: x-quant adds +127 (XFIN), conv epilogues add +128
    beta1 = (f32(4.0) * B1 - a1 * f32(127.0) * K1.astype(np.float32)).astype(
        np.float32
    )
    beta2 = (f32(4.0) * B2 - a2 * f32(128.0) * K2.astype(np.float32)).astype(
        np.float32
    )
    delta0 = B3 * c3s / (f32(127.0) * c2s) + Bs / f32(127.0)
    delta = (
        delta0
        - g3 * (f32(128.0) * K3 + f32(127.0) * Ks).astype(np.float32)
    ).astype(np.float32)

    beta1 = np.ascontiguousarray(beta1.reshape(2, 128).T)  # [128,2]
    beta2 = np.ascontiguousarray(beta2.reshape(2, 128).T)
    delta = np.ascontiguousarray(delta.reshape(8, 128).T)  # [128,8]

    return dict(
        w1l=w1l, w2l=w2l, w3l=w3l, wsl=wsl,
        beta1=beta1, beta2=beta2, delta=delta,
        a1=float(a1), a2=float(a2), g3=float(g3),
    )


def kernel(x, w1, b1, w2, b2, w3, b3, ws, bs):
    global _SCALES
    x = np.asarray(x, dtype=np.float32)
    pre = _prep(
        np.asarray(w1, np.float32), np.asarray(b1, np.float32),
        np.asarray(w2, np.float32), np.asarray(b2, np.float32),
        np.asarray(w3, np.float32), np.asarray(b3, np.float32),
        np.asarray(ws, np.float32), np.asarray(bs, np.float32),
    )
    _SCALES = (pre["a1"], pre["a2"], pre["g3"])
    nc = _build_nc()
    nc.compile()

    shared = {
        "w1l": pre["w1l"], "w2l": pre["w2l"], "w3l": pre["w3l"],
        "wsl": pre["wsl"], "beta1": pre["beta1"], "beta2": pre["beta2"],
        "delta": pre["delta"],
    }
    in_maps = []
    for c in range(N_CORES):
        xs = np.ascontiguousarray(
            x[c * B_LOC : (c + 1) * B_LOC].reshape(B_LOC, 4, 128, 784)
        )
        in_maps.append({"x": xs, **shared})

    import os

    tmpdir = os.environ.get("KERNEL_TRACE_DIR") or None
    if tmpdir:
        os.makedirs(tmpdir, exist_ok=True)
    res = run_bass_kernel_spmd(nc, in_maps, list(range(N_CORES)), tmpdir=tmpdir)
    global LAST_RESULT
    LAST_RESULT = res
    outs = [res.results[c]["y"] for c in range(N_CORES)]
    y = np.concatenate(outs, axis=0)  # [64, 8, 128, 196]
    return np.ascontiguousarray(y.reshape(64, 1024, 14, 14))
